# revision 102
# baseline (speedup 1.0000x reference)
"""BiLSTM-CRF loss kernel for Trainium2 (8 NeuronCores, data-parallel over batch).

Design (per core, B_loc=16 sequences; 5.56ms baseline -> 1.97ms):
  - Forward and backward LSTM directions run INTERLEAVED in a single
    512-iteration loop (iter i: fwd step t=i, bwd step t=511-i) so the two
    independent recurrence chains fill each other's engine stalls.
  - All token embeddings gathered + PE-transposed into a resident s_xT
    buffer in the prologue (shared by both directions).
  - Gate order host-permuted to (i, f, o, g); gate psum split into
    (i,f,o) and (g) tiles so tanh(g) starts during the sigmoid burst.
  - Per step, bias (tiny indicator matmuls) + W_ih@x are PREFETCHED into
    the gate psum one iteration early; only the 16 W_hh matmuls sit on
    the recurrence chain.  ACT reads gates straight from PSUM.
  - c update is unmasked (pad mask is a suffix per sequence; the unfrozen
    c is never read back and stays bounded); h_new written unmasked into
    the h-history slot (frozen-slot garbage is bounded and discarded
    downstream); only st_h is masked via copy_predicated.
  - Batched emission matmuls per 32-step window from the h history; all
    exp() for the CRF batched in the epilogue (no ACT table thrash).
  - CRF log-partition split into alpha (fwd) + beta (bwd) exp-space
    recursions meeting at t=255 -- two ~256-step chains that interleave on
    the engines; alpha's all-active prefix is one fused psum-multiply per
    step.  Rescaling is folded into the next consumed expE slice (off the
    critical path); ln() of the scales deferred to one batched instruction.
  - Gold-path score (unary one-hot reduce + transition row-gather)
    interleaved into the CRF phase on gpsimd/vector.
"""

import numpy as np

PAD_IDX = 0
VOCAB, K, E, H = 30000, 20, 256, 256
B, T = 128, 512
NCORES = 8
BL = B // NCORES          # 16 sequences per core
WIN = 32                  # proj window (time steps)
NW = T // WIN             # 16 windows
RESCALE = 8               # CRF rescale interval

_cache = {}


def _build_program(dt_w, ml=1):
    """Build the SPMD Bass program. dt_w: matmul weight/stream dtype.
    ml: min sequence length over the batch (all-active below this t)."""
    from contextlib import ExitStack
    import concourse.bass as bass
    import concourse.bacc as bacc
    import concourse.tile as tile
    from concourse import mybir
    from concourse.masks import make_identity

    f32 = mybir.dt.float32
    i32 = mybir.dt.int32

    nc = bacc.Bacc(None, target_bir_lowering=False, debug=False)
    names = {}

    with ExitStack() as ctx:
        tc = ctx.enter_context(tile.TileContext(nc))
        dram = ctx.enter_context(tc.tile_pool(name="dram", bufs=1, space="DRAM"))

        def din(key, shape, dt=f32):
            t = dram.tile(shape, dt, kind="ExternalInput", name=key)
            names[key] = t.tensor.name
            return t

        emb = din("emb", [VOCAB, E], dt_w)
        toks = din("toks", [T * BL, 1], i32)          # (w, j, b) window/j-major
        masku = din("masku", [1, T * BL], mybir.dt.uint8)  # col = t*16+b
        tags1h = din("tags1h", [K, T * BL], mybir.dt.uint8)  # one-hot(tag) * mask
        tagsnx = din("tagsnx", [T * BL, K])           # shifted one-hot * mask, f32
        tagsfl = din("tagsfl", [T * BL, 1], i32)      # tag ids, b-major
        wih = {d: din(f"wih_{d}", [E, 4 * H], dt_w) for d in "fb"}
        whh = {d: din(f"whh_{d}", [E, 4 * H], dt_w) for d in "fb"}
        bihT6 = {d: din(f"bihT6_{d}", [6, 128], dt_w) for d in "fb"}
        bihT2 = {d: din(f"bihT2_{d}", [2, 128], dt_w) for d in "fb"}
        indic6 = din("indic6", [6, 96], dt_w)   # indic6[k,(m,b)] = (k==m)
        indic2 = din("indic2", [2, 32], dt_w)
        woutT = din("woutT", [4, 128, K], dt_w)       # chunks: Fk0,Fk1,Bk0,Bk1
        bout = din("bout", [K, 1])
        expAT = din("expAT", [K, K], dt_w)            # exp(transition).T
        expA = din("expA", [K, K], dt_w)              # exp(transition)
        trans = din("trans", [K, K])                  # raw, for row gather
        out_loss = dram.tile([1, BL], f32, kind="ExternalOutput")
        names["out"] = out_loss.tensor.name

        sg = ctx.enter_context(tc.tile_pool(name="sg", bufs=1))       # singles
        tmp = ctx.enter_context(tc.tile_pool(name="tmp", bufs=3))     # step temps
        gat = ctx.enter_context(tc.tile_pool(name="gat", bufs=4))     # gather tiles
        hhp = ctx.enter_context(tc.tile_pool(name="hhp", bufs=2))     # h history
        fin = ctx.enter_context(tc.tile_pool(name="fin", bufs=3))     # finalize
        ps_g = ctx.enter_context(tc.tile_pool(name="ps_g", bufs=2, space="PSUM"))
        ps_e = ctx.enter_context(tc.tile_pool(name="ps_e", bufs=1, space="PSUM"))
        ps_s = ctx.enter_context(tc.tile_pool(name="ps_s", bufs=3, space="PSUM"))

        # ---- resident SBUF tensors ----
        s_wih = {d: sg.tile([128, 2, 4 * H], dt_w, tag=f"wih{d}", name=f"wih{d}") for d in "fb"}
        s_whh = {d: sg.tile([128, 2, 4 * H], dt_w, tag=f"whh{d}", name=f"whh{d}") for d in "fb"}
        for d in "fb":
            nc.sync.dma_start(out=s_wih[d][:], in_=wih[d][:].rearrange("(k p) m -> p k m", p=128))
            nc.sync.dma_start(out=s_whh[d][:], in_=whh[d][:].rearrange("(k p) m -> p k m", p=128))
        s_bihT6 = {d: sg.tile([6, 128], dt_w, tag=f"bihT6{d}", name=f"bihT6{d}") for d in "fb"}
        s_bihT2 = {d: sg.tile([2, 128], dt_w, tag=f"bihT2{d}", name=f"bihT2{d}") for d in "fb"}
        for d in "fb":
            nc.sync.dma_start(out=s_bihT6[d][:], in_=bihT6[d][:])
            nc.sync.dma_start(out=s_bihT2[d][:], in_=bihT2[d][:])
        s_ind6 = sg.tile([6, 96], dt_w, tag="ind6")
        nc.sync.dma_start(out=s_ind6[:], in_=indic6[:])
        s_ind2 = sg.tile([2, 32], dt_w, tag="ind2")
        nc.sync.dma_start(out=s_ind2[:], in_=indic2[:])
        # shared gathered+transposed token features, (p, tw, k, (j b)); both
        # directions project from the same t-window data
        s_xT = sg.tile([128, NW, 2, 512], dt_w, tag="s_xT")
        s_wout = sg.tile([128, 4, K], dt_w, tag="wout")
        nc.sync.dma_start(out=s_wout[:], in_=woutT[:].rearrange("c p k -> p c k"))
        s_bout = sg.tile([K, 1], f32, tag="bout")
        nc.sync.dma_start(out=s_bout[:], in_=bout[:])
        s_expAT = sg.tile([K, K], dt_w, tag="expAT")
        nc.sync.dma_start(out=s_expAT[:], in_=expAT[:])
        s_expA = sg.tile([K, K], dt_w, tag="expA")
        nc.sync.dma_start(out=s_expA[:], in_=expA[:])

        ones = sg.tile([128, K], f32, tag="ones")
        nc.vector.memset(ones[:], 1.0)
        identb = sg.tile([128, 128], dt_w, tag="identb")
        make_identity(nc, identb[:])

        # mask replica: (128, T, BL), col = t*16+b, broadcast across partitions
        maskrep = sg.tile([128, T, BL], mybir.dt.uint8, tag="maskrep")
        nc.sync.dma_start(
            out=maskrep[:],
            in_=bass.AP(tensor=masku.tensor, offset=masku[:].offset,
                        ap=[[0, 128], [BL, T], [1, BL]]),
        )

        emit = sg.tile([K, T, BL], f32, tag="emit")
        expE = sg.tile([K, T, BL], f32, tag="expE")

        # gather indices resident (one upfront DMA each)
        NT128 = T * BL // 128
        idxall = sg.tile([128, NT128], i32, tag="idxall")
        nc.sync.dma_start(out=idxall[:],
                          in_=bass.AP(tensor=toks.tensor, offset=toks[:].offset,
                                      ap=[[1, 128], [128, NT128]]))
        idxtag = sg.tile([128, NT128], i32, tag="idxtag")
        nc.sync.dma_start(out=idxtag[:],
                          in_=bass.AP(tensor=tagsfl.tensor, offset=tagsfl[:].offset,
                                      ap=[[1, 128], [128, NT128]]))
        s_t1h = sg.tile([K, T, BL], mybir.dt.uint8, tag="s_t1h")
        nc.sync.dma_start(out=s_t1h[:].rearrange("k t b -> k (t b)"), in_=tags1h[:])
        s_tnx = sg.tile([128, NT128, K], f32, tag="s_tnx")
        nc.sync.dma_start(out=s_tnx[:],
                          in_=tagsnx[:].rearrange("(n p) k -> p n k", p=128))

        # LSTM states (h in dt_w for matmul rhs, c in f32)
        st_h = {d: sg.tile([128, 2, BL], dt_w, tag=f"h{d}", name=f"h{d}") for d in "fb"}
        st_c = {d: sg.tile([128, 2, BL], f32, tag=f"c{d}", name=f"c{d}") for d in "fb"}
        for d in "fb":
            nc.vector.memset(st_h[d][:], 0.0)
            nc.vector.memset(st_c[d][:], 0.0)

        # CRF alpha/beta state (exp space) + deferred-ln scale buffer
        Bv = sg.tile([K, BL], f32, tag="Bv")
        nc.vector.memset(Bv[:], 1.0)
        Dv = sg.tile([K, BL], dt_w, tag="Dv")   # bf16: feeds the expA matmul
        ones_bf = sg.tile([K, 1], dt_w, tag="ones_bf")
        nc.vector.memset(ones_bf[:], 1.0)
        NRS = T // RESCALE
        sums = sg.tile([1, NRS, BL], f32, tag="sums")
        nc.vector.memset(sums[:], 1.0)

        AF = mybir.ActivationFunctionType
        OP = mybir.AluOpType

        def mask_ap(t, parts, reps):
            """maskrep[:parts, t, :] replicated reps times along a middle dim."""
            base = maskrep[0:parts, t, :]
            if reps == 1:
                return base
            return bass.AP(tensor=base.tensor, offset=base.offset,
                           ap=[base.ap[0], [0, reps], [1, BL]])

        # warm-up matmuls: make PE's clock pass every weight-producing op so
        # steady-state matmuls carry at most one semaphore wait
        for wt in [s_wih["f"][:, 0, 0:1], s_wih["b"][:, 0, 0:1],
                   s_whh["f"][:, 0, 0:1], s_whh["b"][:, 0, 0:1],
                   s_wout[:, 0, 0:1], identb[:, 0:1],
                   s_bihT6["f"][0:1, 0:1], s_bihT6["b"][0:1, 0:1],
                   s_bihT2["f"][0:1, 0:1], s_bihT2["b"][0:1, 0:1]]:
            psd = ps_s.tile([1, 1], f32, tag="pssm")
            nc.tensor.matmul(psd[:], lhsT=wt, rhs=wt, start=True, stop=True)
        psd = ps_s.tile([1, 1], f32, tag="pssm")
        nc.tensor.matmul(psd[:], lhsT=s_expAT[0:K, 0:1], rhs=s_expAT[0:K, 0:1], start=True, stop=True)
        psd = ps_s.tile([1, 1], f32, tag="pssm")
        nc.tensor.matmul(psd[:], lhsT=ones[0:1, 0:1], rhs=ones[0:1, 0:1], start=True, stop=True)

        # ---------- token prep machinery ----------
        # h_hist: (128, 2, WIN, BL) (k, j, b).
        cur_hist = {}

        def prep_thunks(tw):
            """Gather + transpose t-window tw into the shared s_xT buffer."""
            thunks = []
            xg_box = {}

            def gather(g):
                xg = gat.tile([128, E], dt_w, tag="xg", name="xg")
                nc.gpsimd.indirect_dma_start(
                    out=xg[:], out_offset=None, in_=emb[:],
                    in_offset=bass.IndirectOffsetOnAxis(ap=idxall[:, tw * 4 + g:tw * 4 + g + 1], axis=0),
                )
                xg_box[g] = xg

            def tp(g, k):
                pst = ps_e.tile([128, 128], dt_w, tag="pse", name="pst")
                nc.tensor.transpose(out=pst[:], in_=xg_box[g][:, k * 128:(k + 1) * 128], identity=identb[:])
                nc.vector.tensor_copy(s_xT[:, tw, k, g * 128:(g + 1) * 128], pst[:])

            for g in range(4):
                thunks.append(lambda g=g: gather(g))
                for k in range(2):
                    thunks.append(lambda g=g, k=k: tp(g, k))
            return thunks

        def new_hist(d):
            hist = hhp.tile([128, 2, WIN, BL], dt_w, tag=f"hist{d}", name=f"hist{d}")
            return hist

        def emit_window(d, tw, hist, first):
            """Batched emission thunks for t-window tw from hist (ascending t)."""
            cbase = 0 if d == "f" else 2
            box = {}

            def mms():
                pse = ps_e.tile([K, 512], f32, tag="pse", name="pse")
                for k in range(2):
                    nc.tensor.matmul(pse[:], lhsT=s_wout[:, cbase + k, :],
                                     rhs=hist[:, k, :, :], start=(k == 0), stop=(k == 1))
                box["pse"] = pse

            dst = emit[:, tw * WIN:(tw + 1) * WIN, :].rearrange("k t b -> k (t b)")

            def wr(h):
                sl = slice(256 * h, 256 * (h + 1))
                if first:
                    # scalar engine: out = Identity(pse + bias), keeps DVE free
                    nc.scalar.activation(dst[:, sl], box["pse"][:, sl], AF.Identity,
                                         bias=s_bout[:, 0:1])
                else:
                    nc.vector.tensor_tensor(dst[:, sl], box["pse"][:, sl], dst[:, sl], op=OP.add)

            return [mms, lambda: wr(0), lambda: wr(1)]



        # ---------- per-step pieces ----------
        # gate psum split (i,f,o) vs (g).  The bias + W_ih@x part is PREFETCHED
        # in 4-step batches (N=64 matmuls, depends only on s_xT); only the 16
        # W_hh matmuls sit on the recurrence chain, g chunks first so tanh(g)
        # starts during the (i,f,o) burst.  Psum slot order is xcol-ascending:
        # fwd step q of a group uses slot q, bwd uses slot 3-q.
        def gate_prefetch(d, t):
            """Emit bias + input-projection matmuls for step t (off-chain)."""
            psgg = ps_g.tile([128, 2, BL], f32, tag="psgg", name=f"psgg{d}")
            psgi = ps_g.tile([128, 6, BL], f32, tag="psgi", name=f"psgi{d}")
            tw, jj = divmod(t, WIN)
            xcol = slice(jj * BL, (jj + 1) * BL)
            nc.tensor.matmul(psgg[:].rearrange("p m b -> p (m b)"), lhsT=s_bihT2[d][:],
                             rhs=s_ind2[:], start=True, stop=False, skip_group_check=True)
            for m in range(6, 8):
                for k in range(2):
                    nc.tensor.matmul(psgg[:, m - 6], lhsT=s_wih[d][:, k, m * 128:(m + 1) * 128],
                                     rhs=s_xT[:, tw, k, xcol], start=False, stop=False,
                                     skip_group_check=True)
            nc.tensor.matmul(psgi[:].rearrange("p m b -> p (m b)"), lhsT=s_bihT6[d][:],
                             rhs=s_ind6[:], start=True, stop=False, skip_group_check=True)
            for m in range(6):
                for k in range(2):
                    nc.tensor.matmul(psgi[:, m], lhsT=s_wih[d][:, k, m * 128:(m + 1) * 128],
                                     rhs=s_xT[:, tw, k, xcol], start=False, stop=False,
                                     skip_group_check=True)
            return psgi, psgg

        def gate_whh(d, psgi, psgg):
            """Emit the recurrence matmuls for the already-prefetched psums."""
            h = st_h[d]
            for m in range(6, 8):
                for k in range(2):
                    nc.tensor.matmul(psgg[:, m - 6], lhsT=s_whh[d][:, k, m * 128:(m + 1) * 128],
                                     rhs=h[:, k, :], start=False, stop=(m == 7 and k == 1),
                                     skip_group_check=True)
            for m in range(6):
                for k in range(2):
                    nc.tensor.matmul(psgi[:, m], lhsT=s_whh[d][:, k, m * 128:(m + 1) * 128],
                                     rhs=h[:, k, :], start=False, stop=(m == 5 and k == 1),
                                     skip_group_check=True)

        def act_tanh_g(d, psgg):
            gg = tmp.tile([128, 2, BL], f32, tag=f"gg{d}", name=f"gg{d}")
            nc.scalar.activation(gg[:], psgg[:], AF.Tanh)
            return gg

        def act_sig(d, psgi):
            gates = tmp.tile([128, 6, BL], f32, tag=f"gates{d}", name=f"gates{d}")
            nc.scalar.activation(gates[:], psgi[:], AF.Sigmoid)
            return gates

        def cell_mults(d, gates, gg):
            # ig on gpsimd, fc on vector: the two products run on parallel engines
            ig = tmp.tile([128, 2, BL], f32, tag=f"ig{d}", name=f"ig{d}")
            nc.gpsimd.tensor_tensor(ig[:], gates[:, 0:2], gg[:], op=OP.mult)
            fc = tmp.tile([128, 2, BL], f32, tag=f"fc{d}", name=f"fc{d}")
            nc.vector.tensor_tensor(fc[:], gates[:, 2:4], st_c[d][:], op=OP.mult)
            return ig, fc

        def cell_update(d, ig, fc):
            # unmasked c update (frozen-region c is never read back)
            nc.vector.tensor_tensor(st_c[d][:], ig[:], fc[:], op=OP.add)

        def tanh_c(d):
            th = tmp.tile([128, 2, BL], f32, tag=f"th{d}", name=f"th{d}")
            nc.scalar.activation(th[:], st_c[d][:], AF.Tanh)
            return th

        # h_new is written straight into the hist slot (unmasked: frozen-slot
        # garbage is bounded and every consumer discards it); st_h is the only
        # masked state.
        def h_mult(d, gates, th, hist, j):
            nc.vector.tensor_tensor(hist[:, :, j, :], gates[:, 4:6], th[:], op=OP.mult)

        def h_state(d, t, hist, j):
            nc.vector.copy_predicated(st_h[d][:], mask_ap(t, 128, 2), hist[:, :, j, :])

        # ---------- prologue: prep ALL t-windows, prefetch step 0 ----------
        # (prepping mid-loop would serialize the prefetch matmuls behind the
        # s_xT transpose writes via whole-tile dependency tracking)
        for tw in (0, NW - 1, 1, NW - 2, 2, NW - 3, 3, NW - 4,
                   4, NW - 5, 5, NW - 6, 6, NW - 7, 7, 8):
            for th in prep_thunks(tw):
                th()
        cur_hist["f"], cur_hist["b"] = new_hist("f"), new_hist("b")
        prev_hist = {"f": None, "b": None}

        pf = {"f": gate_prefetch("f", 0), "b": gate_prefetch("b", T - 1)}
        pending = []  # emit thunks, drained ~2/iter

        # ---------- main interleaved loop ----------
        for i in range(T):
            blk, j = divmod(i, WIN)
            t_f = i
            t_b = T - 1 - i
            jb = WIN - 1 - j       # bwd hist slot (ascending t within window)

            emit_pending = []
            if j == 0 and blk > 0:
                # windows blk-1 (fwd) and NW-blk (bwd t-window) just completed
                emit_pending += emit_window("f", blk - 1, prev_hist["f"], first=(blk - 1 <= 7))
                emit_pending += emit_window("b", NW - blk, prev_hist["b"], first=(NW - blk >= 8))

            # recurrence matmuls into the prefetched gate psums
            psgi_f, psgg_f = pf["f"]
            psgi_b, psgg_b = pf["b"]
            gate_whh("f", psgi_f, psgg_f)
            gate_whh("b", psgi_b, psgg_b)
            gg_f = act_tanh_g("f", psgg_f)
            g_f = act_sig("f", psgi_f)
            gg_b = act_tanh_g("b", psgg_b)
            ig_f, fc_f = cell_mults("f", g_f, gg_f)
            g_b = act_sig("b", psgi_b)
            cell_update("f", ig_f, fc_f)
            ig_b, fc_b = cell_mults("b", g_b, gg_b)
            th_ff = tanh_c("f")
            cell_update("b", ig_b, fc_b)
            th_bb = tanh_c("b")
            h_mult("f", g_f, th_ff, cur_hist["f"], j)
            h_state("f", t_f, cur_hist["f"], j)
            h_mult("b", g_b, th_bb, cur_hist["b"], jb)
            h_state("b", t_b, cur_hist["b"], jb)

            # prefetch next step's bias + input projection (off-chain PE work)
            if i + 1 < T:
                pf["f"] = gate_prefetch("f", i + 1)
                pf["b"] = gate_prefetch("b", T - 2 - i)

            # emit bookkeeping
            if j == 0:
                pending = list(emit_pending)
                if blk + 1 < NW:
                    h2_f = new_hist("f")
                    h2_b = new_hist("b")
            for _ in range(2):
                if pending:
                    pending.pop(0)()
            if j == WIN - 1:
                while pending:
                    pending.pop(0)()
                prev_hist["f"], prev_hist["b"] = cur_hist["f"], cur_hist["b"]
                if blk + 1 < NW:
                    cur_hist["f"], cur_hist["b"] = h2_f, h2_b

        # ---------- epilogue: last emissions + batched exp ----------
        for th in emit_window("f", NW - 1, prev_hist["f"], first=False):
            th()
        for th in emit_window("b", 0, prev_hist["b"], first=False):
            th()
        for tw in range(NW - 1, -1, -1):
            src = emit[:, tw * WIN:(tw + 1) * WIN, :].rearrange("k t b -> k (t b)")
            dst = expE[:, tw * WIN:(tw + 1) * WIN, :].rearrange("k t b -> k (t b)")
            nc.scalar.activation(dst[:], src[:], AF.Exp)

        # ---------- finalize thunks (interleaved into beta phase) ----------
        fin_thunks = []
        Uacc = fin.tile([K, BL], f32, tag="Uacc")
        nc.vector.memset(Uacc[:], 0.0)
        CH = 32
        TC = T // CH

        def unary_chunk(ci):
            # on gpsimd: the beta phase owns the vector queue
            t1 = fin.tile([K, TC * BL], f32, tag="t1")
            nc.gpsimd.tensor_copy(t1[:], s_t1h[:, ci * TC:(ci + 1) * TC, :].rearrange("p t b -> p (t b)"))
            um = fin.tile([K, TC * BL], f32, tag="um")
            nc.gpsimd.tensor_tensor(
                um[:], t1[:],
                emit[:, ci * TC:(ci + 1) * TC, :].rearrange("p t b -> p (t b)"),
                op=OP.mult)
            ur = fin.tile([K, BL], f32, tag="ur")
            umr = bass.AP(tensor=um.tensor, offset=um[:].offset,
                          ap=[um[:].ap[0], [1, BL], [BL, TC]])
            nc.vector.tensor_reduce(ur[:], umr, axis=mybir.AxisListType.X, op=OP.add)
            nc.gpsimd.tensor_tensor(Uacc[:], Uacc[:], ur[:], op=OP.add)

        for ci in range(CH):
            fin_thunks.append(lambda ci=ci: unary_chunk(ci))

        TRbuf = fin.tile([128, NT128], f32, tag="TRbuf")

        def trans_chunk(i):
            tr = gat.tile([128, K], f32, tag="tr")
            nc.gpsimd.indirect_dma_start(
                out=tr[:], out_offset=None, in_=trans[:],
                in_offset=bass.IndirectOffsetOnAxis(ap=idxtag[:, i:i + 1], axis=0))
            nc.gpsimd.tensor_tensor(tr[:], tr[:], s_tnx[:, i, :], op=OP.mult)
            nc.vector.tensor_reduce(TRbuf[:, i:i + 1], tr[:], axis=mybir.AxisListType.X, op=OP.add)

        for i in range(NT128):
            fin_thunks.append(lambda i=i: trans_chunk(i))

        # ---------- CRF: alpha (fwd) + beta (bwd) recursions, meet at TMID ----
        # Two independent ~256-step chains interleave on the engines.  Alpha's
        # all-active prefix (t < ml) needs just one fused psum-multiply per
        # step.  logZ = ln(sum_i D_mid[i]*B_mid[i]) + deferred ln(scales).
        TMID = T // 2 - 1          # alpha covers t<=TMID, beta covers t>TMID
        # Dv init: D_0 = exp(emit[:, 0, :])
        nc.vector.tensor_copy(Dv[:], expE[:, 0, :])

        def rescale(state, ones_t, fold_t, ri):
            """Normalize state columns; fold the scale into expE[:, fold_t, :]
            (the slice the recursion consumes next) and log the masked sums."""
            pss = ps_s.tile([1, BL], f32, tag="pssm", name="pss")
            nc.tensor.matmul(pss[:], lhsT=ones_t, rhs=state[:], start=True, stop=True)
            nc.vector.copy_predicated(sums[:, ri, :], maskrep[0:1, fold_t, :], pss[:])
            rr = tmp.tile([1, BL], f32, tag="rr")
            nc.vector.reciprocal(rr[:], pss[:])
            psr = ps_s.tile([K, BL], f32, tag="pssm", name="psr")
            nc.tensor.matmul(psr[:], lhsT=ones[0:1, 0:K], rhs=rr[:], start=True, stop=True)
            nc.vector.tensor_tensor(expE[:, fold_t, :], expE[:, fold_t, :], psr[:], op=OP.mult)

        # Emission order per q: [bp_beta, alpha DVE ops, cp_beta] so the two
        # chains overlap instead of alpha queuing behind beta's cp.
        for q in range(T // 2):
            tb = T - 2 - q                   # beta t = 510 .. 255
            ta = 1 + q                       # alpha t = 1 .. 255
            tp1 = tb + 1
            bp = tmp.tile([K, BL], dt_w, tag="bp", name="bp")
            nc.vector.tensor_tensor(bp[:], Bv[:], expE[:, tp1, :], op=OP.mult)
            psb = ps_s.tile([K, BL], f32, tag="pssm", name="psb")
            nc.tensor.matmul(psb[:], lhsT=s_expAT[:], rhs=bp[:], start=True, stop=True)

            if ta <= TMID:
                psd = ps_s.tile([K, BL], f32, tag="pssm", name="psd")
                nc.tensor.matmul(psd[:], lhsT=s_expA[:], rhs=Dv[:], start=True, stop=True)
                if ta < ml:
                    # all sequences active: fused psum-multiply straight into Dv
                    nc.vector.tensor_tensor(Dv[:], psd[:], expE[:, ta, :], op=OP.mult)
                else:
                    dn = tmp.tile([K, BL], dt_w, tag="dn", name="dn")
                    nc.vector.tensor_tensor(dn[:], psd[:], expE[:, ta, :], op=OP.mult)
                    nc.vector.copy_predicated(Dv[:], maskrep[0:K, ta, :], dn[:])

            nc.vector.copy_predicated(Bv[:], maskrep[0:K, tp1, :], psb[:])
            if tb % RESCALE == 0 and tb > TMID:
                rescale(Bv[:], ones[0:K, 0:1], tb, tb // RESCALE - 32)
            if ta <= TMID and ta % RESCALE == 0 and ta < TMID:
                rescale(Dv[:], ones_bf[0:K, 0:1], ta + 1, 32 + ta // RESCALE)
            if fin_thunks and q % 3 == 0:
                fin_thunks.pop(0)()

        while fin_thunks:
            fin_thunks.pop(0)()

        # ---------- final assembly ----------
        dvf = fin.tile([K, BL], f32, tag="dvf")
        nc.vector.tensor_copy(dvf[:], Dv[:])
        zt = fin.tile([K, BL], f32, tag="zt")
        nc.vector.tensor_tensor(zt[:], Bv[:], dvf[:], op=OP.mult)
        psz = ps_s.tile([1, BL], f32, tag="pssm")
        nc.tensor.matmul(psz[:], lhsT=ones[0:K, 0:1], rhs=zt[:], start=True, stop=True)
        logZ = fin.tile([1, BL], f32, tag="logZ")
        nc.scalar.activation(logZ[:], psz[:], AF.Ln)

        # deferred ln of the rescale sums: one batched Ln + strided reduce
        lns = fin.tile([1, NRS, BL], f32, tag="lns")
        nc.scalar.activation(lns[:].rearrange("p r b -> p (r b)"),
                             sums[:].rearrange("p r b -> p (r b)"), AF.Ln)
        lsum = fin.tile([1, BL], f32, tag="lsum")
        lns_ap = bass.AP(tensor=lns.tensor, offset=lns[:].offset,
                         ap=[lns[:].ap[0], [1, BL], [BL, NRS]])
        nc.vector.tensor_reduce(lsum[:], lns_ap, axis=mybir.AxisListType.X, op=OP.add)
        nc.vector.tensor_tensor(logZ[:], logZ[:], lsum[:], op=OP.add)

        # unary total
        psu = ps_s.tile([1, BL], f32, tag="pssm")
        nc.tensor.matmul(psu[:], lhsT=ones[0:K, 0:1], rhs=Uacc[:], start=True, stop=True)
        score = fin.tile([1, BL], f32, tag="score")
        nc.vector.tensor_copy(score[:], psu[:])

        # transition total: colsum TRbuf then per-b strided reduce
        QT = T // 128
        pstr = ps_s.tile([1, NT128], f32, tag="pssm")
        nc.tensor.matmul(pstr[:], lhsT=ones[:, 0:1], rhs=TRbuf[:], start=True, stop=True)
        trv = fin.tile([1, BL], f32, tag="trv")
        ptr_ap = bass.AP(tensor=pstr.tensor, offset=pstr[:].offset,
                         ap=[pstr[:].ap[0], [QT, BL], [1, QT]])
        nc.vector.tensor_reduce(trv[:], ptr_ap, axis=mybir.AxisListType.X, op=OP.add)

        # loss = logZ - (score + trans)
        nc.vector.tensor_tensor(score[:], score[:], trv[:], op=OP.add)
        res = fin.tile([1, BL], f32, tag="res")
        nc.vector.tensor_tensor(res[:], logZ[:], score[:], op=OP.subtract)
        nc.sync.dma_start(out=out_loss[:], in_=res[:])

    nc.compile()
    return nc, names


# torch gate order (i, f, g, o) -> kernel order (i, f, o, g)
def _perm_rows(w):
    return np.concatenate([w[0:2 * H], w[3 * H:4 * H], w[2 * H:3 * H]], axis=0)


def _prep_core(inputs, k, dt_np):
    """Build the per-core input map (host-side index plumbing only)."""
    s = slice(k * BL, (k + 1) * BL)
    sent = np.asarray(inputs["sentences"][s])          # (16, 512) i32
    tags = np.asarray(inputs["tags"][s])               # (16, 512) i32
    mask = (sent != PAD_IDX)
    # toks in (w, j, b) order so gathered/psw columns are (j, b)
    toks = sent.reshape(BL, NW, WIN).transpose(1, 2, 0).reshape(T * BL, 1)
    oh = (tags[:, :, None] == np.arange(K)[None, None, :])
    tags1h = (oh & mask[:, :, None]).transpose(2, 1, 0).reshape(K, T * BL)
    tnx = np.zeros((BL, T, K), np.float32)
    tnx[:, :-1, :] = (oh[:, 1:, :] & mask[:, 1:, None]).astype(np.float32)
    m = {
        "toks": toks.astype(np.int32),
        "masku": mask.T.astype(np.uint8).reshape(1, T * BL),
        "tags1h": tags1h.astype(np.uint8),
        "tagsnx": tnx.reshape(T * BL, K).astype(np.float32),
        "tagsfl": tags.reshape(T * BL, 1).astype(np.int32),
        "emb": np.asarray(inputs["embedding"]).astype(dt_np),
        "wih_f": np.ascontiguousarray(_perm_rows(np.asarray(inputs["w_ih_f"])).T).astype(dt_np),
        "wih_b": np.ascontiguousarray(_perm_rows(np.asarray(inputs["w_ih_b"])).T).astype(dt_np),
        "whh_f": np.ascontiguousarray(_perm_rows(np.asarray(inputs["w_hh_f"])).T).astype(dt_np),
        "whh_b": np.ascontiguousarray(_perm_rows(np.asarray(inputs["w_hh_b"])).T).astype(dt_np),
        "bihT6_f": np.ascontiguousarray(_perm_rows(np.asarray(inputs["b_f"])).reshape(8, 128)[0:6]).astype(dt_np),
        "bihT6_b": np.ascontiguousarray(_perm_rows(np.asarray(inputs["b_b"])).reshape(8, 128)[0:6]).astype(dt_np),
        "bihT2_f": np.ascontiguousarray(_perm_rows(np.asarray(inputs["b_f"])).reshape(8, 128)[6:8]).astype(dt_np),
        "bihT2_b": np.ascontiguousarray(_perm_rows(np.asarray(inputs["b_b"])).reshape(8, 128)[6:8]).astype(dt_np),
        "indic6": (np.arange(6)[:, None] == (np.arange(96) // BL)[None, :]).astype(dt_np),
        "indic2": (np.arange(2)[:, None] == (np.arange(32) // BL)[None, :]).astype(dt_np),
        "woutT": np.ascontiguousarray(np.asarray(inputs["w_out"]).T.reshape(4, 128, K)).astype(dt_np),
        "bout": np.asarray(inputs["b_out"]).reshape(K, 1).astype(np.float32),
        "expAT": np.ascontiguousarray(np.exp(np.asarray(inputs["transition"], np.float64)).T).astype(dt_np),
        "expA": np.exp(np.asarray(inputs["transition"], np.float64)).astype(dt_np),
        "trans": np.asarray(inputs["transition"], np.float32),
    }
    return m


def kernel(**inputs):
    import ml_dtypes
    from concourse import mybir
    from concourse.bass_utils import run_bass_kernel_spmd

    use_bf16 = _cache.get("use_bf16", True)
    ml = max(1, int(np.asarray(inputs["sentences_lengths"]).min()))
    key = ("prog", use_bf16, ml)
    if key not in _cache:
        dt_w = mybir.dt.bfloat16 if use_bf16 else mybir.dt.float32
        _cache[key] = _build_program(dt_w, ml)
    nc, names = _cache[key]
    dt_np = ml_dtypes.bfloat16 if use_bf16 else np.float32

    in_maps = []
    for k in range(NCORES):
        m = _prep_core(inputs, k, dt_np)
        in_maps.append({names[kk]: vv for kk, vv in m.items()})

    res = run_bass_kernel_spmd(nc, in_maps, core_ids=list(range(NCORES)),
                               **_cache.get("run_kwargs", {}))
    out = np.concatenate([r[names["out"]].reshape(BL) for r in res.results])
    _cache["last_results"] = res
    return out.astype(np.float32)


# revision 104
# speedup vs baseline: 1.0016x; 1.0016x over previous
"""BiLSTM-CRF loss kernel for Trainium2 (8 NeuronCores, data-parallel over batch).

Design (per core, B_loc=16 sequences; 5.56ms baseline -> 1.97ms):
  - Forward and backward LSTM directions run INTERLEAVED in a single
    512-iteration loop (iter i: fwd step t=i, bwd step t=511-i) so the two
    independent recurrence chains fill each other's engine stalls.
  - All token embeddings gathered + PE-transposed into a resident s_xT
    buffer in the prologue (shared by both directions).
  - Gate order host-permuted to (i, f, o, g); gate psum split into
    (i,f,o) and (g) tiles so tanh(g) starts during the sigmoid burst.
  - Per step, bias (tiny indicator matmuls) + W_ih@x are PREFETCHED into
    the gate psum one iteration early; only the 16 W_hh matmuls sit on
    the recurrence chain.  ACT reads gates straight from PSUM.
  - c update is unmasked (pad mask is a suffix per sequence; the unfrozen
    c is never read back and stays bounded); h_new written unmasked into
    the h-history slot (frozen-slot garbage is bounded and discarded
    downstream); only st_h is masked via copy_predicated.
  - Batched emission matmuls per 32-step window from the h history; all
    exp() for the CRF batched in the epilogue (no ACT table thrash).
  - CRF log-partition split into alpha (fwd) + beta (bwd) exp-space
    recursions meeting at t=255 -- two ~256-step chains that interleave on
    the engines; alpha's all-active prefix is one fused psum-multiply per
    step.  Rescaling is folded into the next consumed expE slice (off the
    critical path); ln() of the scales deferred to one batched instruction.
  - Gold-path score (unary one-hot reduce + transition row-gather)
    interleaved into the CRF phase on gpsimd/vector.
"""

import numpy as np

PAD_IDX = 0
VOCAB, K, E, H = 30000, 20, 256, 256
B, T = 128, 512
NCORES = 8
BL = B // NCORES          # 16 sequences per core
WIN = 32                  # proj window (time steps)
NW = T // WIN             # 16 windows
RESCALE = 8               # CRF rescale interval

_cache = {}


def _build_program(dt_w, ml=1):
    """Build the SPMD Bass program. dt_w: matmul weight/stream dtype.
    ml: min sequence length over the batch (all-active below this t)."""
    from contextlib import ExitStack
    import concourse.bass as bass
    import concourse.bacc as bacc
    import concourse.tile as tile
    from concourse import mybir
    from concourse.masks import make_identity

    f32 = mybir.dt.float32
    i32 = mybir.dt.int32

    nc = bacc.Bacc(None, target_bir_lowering=False, debug=False)
    names = {}

    with ExitStack() as ctx:
        tc = ctx.enter_context(tile.TileContext(nc))
        dram = ctx.enter_context(tc.tile_pool(name="dram", bufs=1, space="DRAM"))

        def din(key, shape, dt=f32):
            t = dram.tile(shape, dt, kind="ExternalInput", name=key)
            names[key] = t.tensor.name
            return t

        emb = din("emb", [VOCAB, E], dt_w)
        toks = din("toks", [T * BL, 1], i32)          # (w, j, b) window/j-major
        masku = din("masku", [1, T * BL], mybir.dt.uint8)  # col = t*16+b
        tags1h = din("tags1h", [K, T * BL], mybir.dt.uint8)  # one-hot(tag) * mask
        tagsnx = din("tagsnx", [T * BL, K])           # shifted one-hot * mask, f32
        tagsfl = din("tagsfl", [T * BL, 1], i32)      # tag ids, b-major
        wih = {d: din(f"wih_{d}", [E, 4 * H], dt_w) for d in "fb"}
        whh = {d: din(f"whh_{d}", [E, 4 * H], dt_w) for d in "fb"}
        bihT6 = {d: din(f"bihT6_{d}", [6, 128], dt_w) for d in "fb"}
        bihT2 = {d: din(f"bihT2_{d}", [2, 128], dt_w) for d in "fb"}
        indic6 = din("indic6", [6, 96], dt_w)   # indic6[k,(m,b)] = (k==m)
        indic2 = din("indic2", [2, 32], dt_w)
        woutT = din("woutT", [4, 128, K], dt_w)       # chunks: Fk0,Fk1,Bk0,Bk1
        bout = din("bout", [K, 1])
        expAT = din("expAT", [K, K], dt_w)            # exp(transition).T
        expA = din("expA", [K, K], dt_w)              # exp(transition)
        trans = din("trans", [K, K])                  # raw, for row gather
        out_loss = dram.tile([1, BL], f32, kind="ExternalOutput")
        names["out"] = out_loss.tensor.name

        sg = ctx.enter_context(tc.tile_pool(name="sg", bufs=1))       # singles
        tmp = ctx.enter_context(tc.tile_pool(name="tmp", bufs=3))     # step temps
        gat = ctx.enter_context(tc.tile_pool(name="gat", bufs=4))     # gather tiles
        hhp = ctx.enter_context(tc.tile_pool(name="hhp", bufs=2))     # h history
        fin = ctx.enter_context(tc.tile_pool(name="fin", bufs=3))     # finalize
        ps_g = ctx.enter_context(tc.tile_pool(name="ps_g", bufs=2, space="PSUM"))
        ps_e = ctx.enter_context(tc.tile_pool(name="ps_e", bufs=1, space="PSUM"))
        ps_s = ctx.enter_context(tc.tile_pool(name="ps_s", bufs=3, space="PSUM"))

        # ---- resident SBUF tensors ----
        s_wih = {d: sg.tile([128, 2, 4 * H], dt_w, tag=f"wih{d}", name=f"wih{d}") for d in "fb"}
        s_whh = {d: sg.tile([128, 2, 4 * H], dt_w, tag=f"whh{d}", name=f"whh{d}") for d in "fb"}
        for d in "fb":
            nc.sync.dma_start(out=s_wih[d][:], in_=wih[d][:].rearrange("(k p) m -> p k m", p=128))
            nc.sync.dma_start(out=s_whh[d][:], in_=whh[d][:].rearrange("(k p) m -> p k m", p=128))
        s_bihT6 = {d: sg.tile([6, 128], dt_w, tag=f"bihT6{d}", name=f"bihT6{d}") for d in "fb"}
        s_bihT2 = {d: sg.tile([2, 128], dt_w, tag=f"bihT2{d}", name=f"bihT2{d}") for d in "fb"}
        for d in "fb":
            nc.sync.dma_start(out=s_bihT6[d][:], in_=bihT6[d][:])
            nc.sync.dma_start(out=s_bihT2[d][:], in_=bihT2[d][:])
        s_ind6 = sg.tile([6, 96], dt_w, tag="ind6")
        nc.sync.dma_start(out=s_ind6[:], in_=indic6[:])
        s_ind2 = sg.tile([2, 32], dt_w, tag="ind2")
        nc.sync.dma_start(out=s_ind2[:], in_=indic2[:])
        # shared gathered+transposed token features, (p, tw, k, (j b)); both
        # directions project from the same t-window data
        s_xT = sg.tile([128, NW, 2, 512], dt_w, tag="s_xT")
        s_wout = sg.tile([128, 4, K], dt_w, tag="wout")
        nc.sync.dma_start(out=s_wout[:], in_=woutT[:].rearrange("c p k -> p c k"))
        s_bout = sg.tile([K, 1], f32, tag="bout")
        nc.sync.dma_start(out=s_bout[:], in_=bout[:])
        s_expAT = sg.tile([K, K], dt_w, tag="expAT")
        nc.sync.dma_start(out=s_expAT[:], in_=expAT[:])
        s_expA = sg.tile([K, K], dt_w, tag="expA")
        nc.sync.dma_start(out=s_expA[:], in_=expA[:])

        ones = sg.tile([128, K], f32, tag="ones")
        nc.vector.memset(ones[:], 1.0)
        identb = sg.tile([128, 128], dt_w, tag="identb")
        make_identity(nc, identb[:])

        # mask replica: (128, T, BL), col = t*16+b, broadcast across partitions
        maskrep = sg.tile([128, T, BL], mybir.dt.uint8, tag="maskrep")
        nc.sync.dma_start(
            out=maskrep[:],
            in_=bass.AP(tensor=masku.tensor, offset=masku[:].offset,
                        ap=[[0, 128], [BL, T], [1, BL]]),
        )

        emit = sg.tile([K, T, BL], f32, tag="emit")
        expE = sg.tile([K, T, BL], f32, tag="expE")

        # gather indices resident (one upfront DMA each)
        NT128 = T * BL // 128
        idxall = sg.tile([128, NT128], i32, tag="idxall")
        nc.sync.dma_start(out=idxall[:],
                          in_=bass.AP(tensor=toks.tensor, offset=toks[:].offset,
                                      ap=[[1, 128], [128, NT128]]))
        idxtag = sg.tile([128, NT128], i32, tag="idxtag")
        nc.sync.dma_start(out=idxtag[:],
                          in_=bass.AP(tensor=tagsfl.tensor, offset=tagsfl[:].offset,
                                      ap=[[1, 128], [128, NT128]]))
        s_t1h = sg.tile([K, T, BL], mybir.dt.uint8, tag="s_t1h")
        nc.sync.dma_start(out=s_t1h[:].rearrange("k t b -> k (t b)"), in_=tags1h[:])
        s_tnx = sg.tile([128, NT128, K], f32, tag="s_tnx")
        nc.sync.dma_start(out=s_tnx[:],
                          in_=tagsnx[:].rearrange("(n p) k -> p n k", p=128))

        # LSTM states (h in dt_w for matmul rhs, c in f32)
        st_h = {d: sg.tile([128, 2, BL], dt_w, tag=f"h{d}", name=f"h{d}") for d in "fb"}
        st_c = {d: sg.tile([128, 2, BL], f32, tag=f"c{d}", name=f"c{d}") for d in "fb"}
        for d in "fb":
            nc.vector.memset(st_h[d][:], 0.0)
            nc.vector.memset(st_c[d][:], 0.0)

        # CRF alpha/beta state (exp space) + deferred-ln scale buffer
        Bv = sg.tile([K, BL], f32, tag="Bv")
        nc.vector.memset(Bv[:], 1.0)
        Dv = sg.tile([K, BL], dt_w, tag="Dv")   # bf16: feeds the expA matmul
        ones_bf = sg.tile([K, 1], dt_w, tag="ones_bf")
        nc.vector.memset(ones_bf[:], 1.0)
        NRS = T // RESCALE
        sums = sg.tile([1, NRS, BL], f32, tag="sums")
        nc.vector.memset(sums[:], 1.0)

        AF = mybir.ActivationFunctionType
        OP = mybir.AluOpType

        def mask_ap(t, parts, reps):
            """maskrep[:parts, t, :] replicated reps times along a middle dim."""
            base = maskrep[0:parts, t, :]
            if reps == 1:
                return base
            return bass.AP(tensor=base.tensor, offset=base.offset,
                           ap=[base.ap[0], [0, reps], [1, BL]])

        # warm-up matmuls: make PE's clock pass every weight-producing op so
        # steady-state matmuls carry at most one semaphore wait
        for wt in [s_wih["f"][:, 0, 0:1], s_wih["b"][:, 0, 0:1],
                   s_whh["f"][:, 0, 0:1], s_whh["b"][:, 0, 0:1],
                   s_wout[:, 0, 0:1], identb[:, 0:1],
                   s_bihT6["f"][0:1, 0:1], s_bihT6["b"][0:1, 0:1],
                   s_bihT2["f"][0:1, 0:1], s_bihT2["b"][0:1, 0:1]]:
            psd = ps_s.tile([1, 1], f32, tag="pssm")
            nc.tensor.matmul(psd[:], lhsT=wt, rhs=wt, start=True, stop=True)
        psd = ps_s.tile([1, 1], f32, tag="pssm")
        nc.tensor.matmul(psd[:], lhsT=s_expAT[0:K, 0:1], rhs=s_expAT[0:K, 0:1], start=True, stop=True)
        psd = ps_s.tile([1, 1], f32, tag="pssm")
        nc.tensor.matmul(psd[:], lhsT=ones[0:1, 0:1], rhs=ones[0:1, 0:1], start=True, stop=True)

        # ---------- token prep machinery ----------
        # h_hist: (128, 2, WIN, BL) (k, j, b).
        cur_hist = {}

        def prep_thunks(tw):
            """Gather + transpose t-window tw into the shared s_xT buffer."""
            thunks = []
            xg_box = {}

            def gather(g):
                xg = gat.tile([128, E], dt_w, tag="xg", name="xg")
                nc.gpsimd.indirect_dma_start(
                    out=xg[:], out_offset=None, in_=emb[:],
                    in_offset=bass.IndirectOffsetOnAxis(ap=idxall[:, tw * 4 + g:tw * 4 + g + 1], axis=0),
                )
                xg_box[g] = xg

            def tp(g, k):
                pst = ps_e.tile([128, 128], dt_w, tag="pse", name="pst")
                nc.tensor.transpose(out=pst[:], in_=xg_box[g][:, k * 128:(k + 1) * 128], identity=identb[:])
                nc.vector.tensor_copy(s_xT[:, tw, k, g * 128:(g + 1) * 128], pst[:])

            for g in range(4):
                thunks.append(lambda g=g: gather(g))
                for k in range(2):
                    thunks.append(lambda g=g, k=k: tp(g, k))
            return thunks

        def new_hist(d):
            hist = hhp.tile([128, 2, WIN, BL], dt_w, tag=f"hist{d}", name=f"hist{d}")
            return hist

        def emit_window(d, tw, hist, first):
            """Batched emission thunks for t-window tw from hist (ascending t).
            Split into single-matmul / half-width-write thunks so the drain
            never puts a fat op right in front of the chain-critical queues."""
            cbase = 0 if d == "f" else 2
            box = {}

            def mm(h, k):
                if "pse" not in box:
                    box["pse"] = ps_e.tile([K, 512], f32, tag="pse", name="pse")
                sl = slice(256 * h, 256 * (h + 1))
                nc.tensor.matmul(box["pse"][:, sl], lhsT=s_wout[:, cbase + k, :],
                                 rhs=hist[:, k, :, :].rearrange("p t b -> p (t b)")[:, sl],
                                 start=(k == 0), stop=(k == 1))

            dst = emit[:, tw * WIN:(tw + 1) * WIN, :].rearrange("k t b -> k (t b)")

            def wr(h):
                sl = slice(256 * h, 256 * (h + 1))
                if first:
                    # scalar engine: out = Identity(pse + bias), keeps DVE free
                    nc.scalar.activation(dst[:, sl], box["pse"][:, sl], AF.Identity,
                                         bias=s_bout[:, 0:1])
                else:
                    nc.vector.tensor_tensor(dst[:, sl], box["pse"][:, sl], dst[:, sl], op=OP.add)

            return [lambda: mm(0, 0), lambda: mm(0, 1), lambda: mm(1, 0),
                    lambda: mm(1, 1), lambda: wr(0), lambda: wr(1)]



        # ---------- per-step pieces ----------
        # gate psum split (i,f,o) vs (g).  The bias + W_ih@x part is PREFETCHED
        # in 4-step batches (N=64 matmuls, depends only on s_xT); only the 16
        # W_hh matmuls sit on the recurrence chain, g chunks first so tanh(g)
        # starts during the (i,f,o) burst.  Psum slot order is xcol-ascending:
        # fwd step q of a group uses slot q, bwd uses slot 3-q.
        def gate_prefetch(d, t):
            """Emit bias + input-projection matmuls for step t (off-chain)."""
            psgg = ps_g.tile([128, 2, BL], f32, tag="psgg", name=f"psgg{d}")
            psgi = ps_g.tile([128, 6, BL], f32, tag="psgi", name=f"psgi{d}")
            tw, jj = divmod(t, WIN)
            xcol = slice(jj * BL, (jj + 1) * BL)
            nc.tensor.matmul(psgg[:].rearrange("p m b -> p (m b)"), lhsT=s_bihT2[d][:],
                             rhs=s_ind2[:], start=True, stop=False, skip_group_check=True)
            for m in range(6, 8):
                for k in range(2):
                    nc.tensor.matmul(psgg[:, m - 6], lhsT=s_wih[d][:, k, m * 128:(m + 1) * 128],
                                     rhs=s_xT[:, tw, k, xcol], start=False, stop=False,
                                     skip_group_check=True)
            nc.tensor.matmul(psgi[:].rearrange("p m b -> p (m b)"), lhsT=s_bihT6[d][:],
                             rhs=s_ind6[:], start=True, stop=False, skip_group_check=True)
            for m in range(6):
                for k in range(2):
                    nc.tensor.matmul(psgi[:, m], lhsT=s_wih[d][:, k, m * 128:(m + 1) * 128],
                                     rhs=s_xT[:, tw, k, xcol], start=False, stop=False,
                                     skip_group_check=True)
            return psgi, psgg

        def gate_whh(d, psgi, psgg):
            """Emit the recurrence matmuls for the already-prefetched psums."""
            h = st_h[d]
            for m in range(6, 8):
                for k in range(2):
                    nc.tensor.matmul(psgg[:, m - 6], lhsT=s_whh[d][:, k, m * 128:(m + 1) * 128],
                                     rhs=h[:, k, :], start=False, stop=(m == 7 and k == 1),
                                     skip_group_check=True)
            for m in range(6):
                for k in range(2):
                    nc.tensor.matmul(psgi[:, m], lhsT=s_whh[d][:, k, m * 128:(m + 1) * 128],
                                     rhs=h[:, k, :], start=False, stop=(m == 5 and k == 1),
                                     skip_group_check=True)

        def act_tanh_g(d, psgg):
            gg = tmp.tile([128, 2, BL], f32, tag=f"gg{d}", name=f"gg{d}")
            nc.scalar.activation(gg[:], psgg[:], AF.Tanh)
            return gg

        def act_sig(d, psgi):
            gates = tmp.tile([128, 6, BL], f32, tag=f"gates{d}", name=f"gates{d}")
            nc.scalar.activation(gates[:], psgi[:], AF.Sigmoid)
            return gates

        def cell_mults(d, gates, gg):
            # ig on gpsimd, fc on vector: the two products run on parallel engines
            ig = tmp.tile([128, 2, BL], f32, tag=f"ig{d}", name=f"ig{d}")
            nc.gpsimd.tensor_tensor(ig[:], gates[:, 0:2], gg[:], op=OP.mult)
            fc = tmp.tile([128, 2, BL], f32, tag=f"fc{d}", name=f"fc{d}")
            nc.vector.tensor_tensor(fc[:], gates[:, 2:4], st_c[d][:], op=OP.mult)
            return ig, fc

        def cell_update(d, ig, fc):
            # unmasked c update (frozen-region c is never read back)
            nc.vector.tensor_tensor(st_c[d][:], ig[:], fc[:], op=OP.add)

        def tanh_c(d):
            th = tmp.tile([128, 2, BL], f32, tag=f"th{d}", name=f"th{d}")
            nc.scalar.activation(th[:], st_c[d][:], AF.Tanh)
            return th

        # h_new is written straight into the hist slot (unmasked: frozen-slot
        # garbage is bounded and every consumer discards it); st_h is the only
        # masked state.
        def h_mult(d, gates, th, hist, j):
            nc.vector.tensor_tensor(hist[:, :, j, :], gates[:, 4:6], th[:], op=OP.mult)

        def h_state(d, t, hist, j):
            nc.vector.copy_predicated(st_h[d][:], mask_ap(t, 128, 2), hist[:, :, j, :])

        # ---------- prologue: prep ALL t-windows, prefetch step 0 ----------
        # (prepping mid-loop would serialize the prefetch matmuls behind the
        # s_xT transpose writes via whole-tile dependency tracking)
        for tw in (0, NW - 1, 1, NW - 2, 2, NW - 3, 3, NW - 4,
                   4, NW - 5, 5, NW - 6, 6, NW - 7, 7, 8):
            for th in prep_thunks(tw):
                th()
        cur_hist["f"], cur_hist["b"] = new_hist("f"), new_hist("b")
        prev_hist = {"f": None, "b": None}

        pf = {"f": gate_prefetch("f", 0), "b": gate_prefetch("b", T - 1)}
        pending = []  # emit thunks, drained ~2/iter

        # ---------- main interleaved loop ----------
        for i in range(T):
            blk, j = divmod(i, WIN)
            t_f = i
            t_b = T - 1 - i
            jb = WIN - 1 - j       # bwd hist slot (ascending t within window)

            emit_pending = []
            if j == 0 and blk > 0:
                # windows blk-1 (fwd) and NW-blk (bwd t-window) just completed
                emit_pending += emit_window("f", blk - 1, prev_hist["f"], first=(blk - 1 <= 7))
                emit_pending += emit_window("b", NW - blk, prev_hist["b"], first=(NW - blk >= 8))

            # recurrence matmuls into the prefetched gate psums
            psgi_f, psgg_f = pf["f"]
            psgi_b, psgg_b = pf["b"]
            gate_whh("f", psgi_f, psgg_f)
            gate_whh("b", psgi_b, psgg_b)
            gg_f = act_tanh_g("f", psgg_f)
            g_f = act_sig("f", psgi_f)
            gg_b = act_tanh_g("b", psgg_b)
            ig_f, fc_f = cell_mults("f", g_f, gg_f)
            g_b = act_sig("b", psgi_b)
            cell_update("f", ig_f, fc_f)
            ig_b, fc_b = cell_mults("b", g_b, gg_b)
            th_ff = tanh_c("f")
            cell_update("b", ig_b, fc_b)
            th_bb = tanh_c("b")
            h_mult("f", g_f, th_ff, cur_hist["f"], j)
            h_state("f", t_f, cur_hist["f"], j)
            h_mult("b", g_b, th_bb, cur_hist["b"], jb)
            h_state("b", t_b, cur_hist["b"], jb)

            # prefetch next step's bias + input projection (off-chain PE work)
            if i + 1 < T:
                pf["f"] = gate_prefetch("f", i + 1)
                pf["b"] = gate_prefetch("b", T - 2 - i)

            # emit bookkeeping
            if j == 0:
                pending = list(emit_pending)
                if blk + 1 < NW:
                    h2_f = new_hist("f")
                    h2_b = new_hist("b")
            for _ in range(2):
                if pending:
                    pending.pop(0)()
            if j == WIN - 1:
                while pending:
                    pending.pop(0)()
                prev_hist["f"], prev_hist["b"] = cur_hist["f"], cur_hist["b"]
                if blk + 1 < NW:
                    cur_hist["f"], cur_hist["b"] = h2_f, h2_b

        # ---------- epilogue: last emissions + batched exp ----------
        for th in emit_window("f", NW - 1, prev_hist["f"], first=False):
            th()
        for th in emit_window("b", 0, prev_hist["b"], first=False):
            th()
        for tw in range(NW - 1, -1, -1):
            src = emit[:, tw * WIN:(tw + 1) * WIN, :].rearrange("k t b -> k (t b)")
            dst = expE[:, tw * WIN:(tw + 1) * WIN, :].rearrange("k t b -> k (t b)")
            nc.scalar.activation(dst[:], src[:], AF.Exp)

        # ---------- finalize thunks (interleaved into beta phase) ----------
        fin_thunks = []
        Uacc = fin.tile([K, BL], f32, tag="Uacc")
        nc.vector.memset(Uacc[:], 0.0)
        CH = 32
        TC = T // CH

        def unary_chunk(ci):
            # on gpsimd: the beta phase owns the vector queue
            t1 = fin.tile([K, TC * BL], f32, tag="t1")
            nc.gpsimd.tensor_copy(t1[:], s_t1h[:, ci * TC:(ci + 1) * TC, :].rearrange("p t b -> p (t b)"))
            um = fin.tile([K, TC * BL], f32, tag="um")
            nc.gpsimd.tensor_tensor(
                um[:], t1[:],
                emit[:, ci * TC:(ci + 1) * TC, :].rearrange("p t b -> p (t b)"),
                op=OP.mult)
            ur = fin.tile([K, BL], f32, tag="ur")
            umr = bass.AP(tensor=um.tensor, offset=um[:].offset,
                          ap=[um[:].ap[0], [1, BL], [BL, TC]])
            nc.vector.tensor_reduce(ur[:], umr, axis=mybir.AxisListType.X, op=OP.add)
            nc.gpsimd.tensor_tensor(Uacc[:], Uacc[:], ur[:], op=OP.add)

        for ci in range(CH):
            fin_thunks.append(lambda ci=ci: unary_chunk(ci))

        TRbuf = fin.tile([128, NT128], f32, tag="TRbuf")

        def trans_chunk(i):
            tr = gat.tile([128, K], f32, tag="tr")
            nc.gpsimd.indirect_dma_start(
                out=tr[:], out_offset=None, in_=trans[:],
                in_offset=bass.IndirectOffsetOnAxis(ap=idxtag[:, i:i + 1], axis=0))
            nc.gpsimd.tensor_tensor(tr[:], tr[:], s_tnx[:, i, :], op=OP.mult)
            nc.vector.tensor_reduce(TRbuf[:, i:i + 1], tr[:], axis=mybir.AxisListType.X, op=OP.add)

        for i in range(NT128):
            fin_thunks.append(lambda i=i: trans_chunk(i))

        # ---------- CRF: alpha (fwd) + beta (bwd) recursions, meet at TMID ----
        # Two independent ~256-step chains interleave on the engines.  Alpha's
        # all-active prefix (t < ml) needs just one fused psum-multiply per
        # step.  logZ = ln(sum_i D_mid[i]*B_mid[i]) + deferred ln(scales).
        TMID = T // 2 - 1          # alpha covers t<=TMID, beta covers t>TMID
        # Dv init: D_0 = exp(emit[:, 0, :])
        nc.vector.tensor_copy(Dv[:], expE[:, 0, :])

        def rescale(state, ones_t, fold_t, ri):
            """Normalize state columns; fold the scale into expE[:, fold_t, :]
            (the slice the recursion consumes next) and log the masked sums."""
            pss = ps_s.tile([1, BL], f32, tag="pssm", name="pss")
            nc.tensor.matmul(pss[:], lhsT=ones_t, rhs=state[:], start=True, stop=True)
            nc.vector.copy_predicated(sums[:, ri, :], maskrep[0:1, fold_t, :], pss[:])
            rr = tmp.tile([1, BL], f32, tag="rr")
            nc.vector.reciprocal(rr[:], pss[:])
            psr = ps_s.tile([K, BL], f32, tag="pssm", name="psr")
            nc.tensor.matmul(psr[:], lhsT=ones[0:1, 0:K], rhs=rr[:], start=True, stop=True)
            nc.vector.tensor_tensor(expE[:, fold_t, :], expE[:, fold_t, :], psr[:], op=OP.mult)

        # Emission order per q: [bp_beta, alpha DVE ops, cp_beta] so the two
        # chains overlap instead of alpha queuing behind beta's cp.
        for q in range(T // 2):
            tb = T - 2 - q                   # beta t = 510 .. 255
            ta = 1 + q                       # alpha t = 1 .. 255
            tp1 = tb + 1
            bp = tmp.tile([K, BL], dt_w, tag="bp", name="bp")
            nc.vector.tensor_tensor(bp[:], Bv[:], expE[:, tp1, :], op=OP.mult)
            psb = ps_s.tile([K, BL], f32, tag="pssm", name="psb")
            nc.tensor.matmul(psb[:], lhsT=s_expAT[:], rhs=bp[:], start=True, stop=True)

            if ta <= TMID:
                psd = ps_s.tile([K, BL], f32, tag="pssm", name="psd")
                nc.tensor.matmul(psd[:], lhsT=s_expA[:], rhs=Dv[:], start=True, stop=True)
                if ta < ml:
                    # all sequences active: fused psum-multiply straight into Dv
                    nc.vector.tensor_tensor(Dv[:], psd[:], expE[:, ta, :], op=OP.mult)
                else:
                    dn = tmp.tile([K, BL], dt_w, tag="dn", name="dn")
                    nc.vector.tensor_tensor(dn[:], psd[:], expE[:, ta, :], op=OP.mult)
                    nc.vector.copy_predicated(Dv[:], maskrep[0:K, ta, :], dn[:])

            nc.vector.copy_predicated(Bv[:], maskrep[0:K, tp1, :], psb[:])
            if tb % RESCALE == 0 and tb > TMID:
                rescale(Bv[:], ones[0:K, 0:1], tb, tb // RESCALE - 32)
            if ta <= TMID and ta % RESCALE == 0 and ta < TMID:
                rescale(Dv[:], ones_bf[0:K, 0:1], ta + 1, 32 + ta // RESCALE)
            if fin_thunks and q % 2 == 0:
                fin_thunks.pop(0)()

        while fin_thunks:
            fin_thunks.pop(0)()

        # ---------- final assembly ----------
        dvf = fin.tile([K, BL], f32, tag="dvf")
        nc.vector.tensor_copy(dvf[:], Dv[:])
        zt = fin.tile([K, BL], f32, tag="zt")
        nc.vector.tensor_tensor(zt[:], Bv[:], dvf[:], op=OP.mult)
        psz = ps_s.tile([1, BL], f32, tag="pssm")
        nc.tensor.matmul(psz[:], lhsT=ones[0:K, 0:1], rhs=zt[:], start=True, stop=True)
        logZ = fin.tile([1, BL], f32, tag="logZ")
        nc.scalar.activation(logZ[:], psz[:], AF.Ln)

        # deferred ln of the rescale sums: one batched Ln + strided reduce
        lns = fin.tile([1, NRS, BL], f32, tag="lns")
        nc.scalar.activation(lns[:].rearrange("p r b -> p (r b)"),
                             sums[:].rearrange("p r b -> p (r b)"), AF.Ln)
        lsum = fin.tile([1, BL], f32, tag="lsum")
        lns_ap = bass.AP(tensor=lns.tensor, offset=lns[:].offset,
                         ap=[lns[:].ap[0], [1, BL], [BL, NRS]])
        nc.vector.tensor_reduce(lsum[:], lns_ap, axis=mybir.AxisListType.X, op=OP.add)
        nc.vector.tensor_tensor(logZ[:], logZ[:], lsum[:], op=OP.add)

        # unary total
        psu = ps_s.tile([1, BL], f32, tag="pssm")
        nc.tensor.matmul(psu[:], lhsT=ones[0:K, 0:1], rhs=Uacc[:], start=True, stop=True)
        score = fin.tile([1, BL], f32, tag="score")
        nc.vector.tensor_copy(score[:], psu[:])

        # transition total: colsum TRbuf then per-b strided reduce
        QT = T // 128
        pstr = ps_s.tile([1, NT128], f32, tag="pssm")
        nc.tensor.matmul(pstr[:], lhsT=ones[:, 0:1], rhs=TRbuf[:], start=True, stop=True)
        trv = fin.tile([1, BL], f32, tag="trv")
        ptr_ap = bass.AP(tensor=pstr.tensor, offset=pstr[:].offset,
                         ap=[pstr[:].ap[0], [QT, BL], [1, QT]])
        nc.vector.tensor_reduce(trv[:], ptr_ap, axis=mybir.AxisListType.X, op=OP.add)

        # loss = logZ - (score + trans)
        nc.vector.tensor_tensor(score[:], score[:], trv[:], op=OP.add)
        res = fin.tile([1, BL], f32, tag="res")
        nc.vector.tensor_tensor(res[:], logZ[:], score[:], op=OP.subtract)
        nc.sync.dma_start(out=out_loss[:], in_=res[:])

    nc.compile()
    return nc, names


# torch gate order (i, f, g, o) -> kernel order (i, f, o, g)
def _perm_rows(w):
    return np.concatenate([w[0:2 * H], w[3 * H:4 * H], w[2 * H:3 * H]], axis=0)


def _prep_core(inputs, k, dt_np):
    """Build the per-core input map (host-side index plumbing only)."""
    s = slice(k * BL, (k + 1) * BL)
    sent = np.asarray(inputs["sentences"][s])          # (16, 512) i32
    tags = np.asarray(inputs["tags"][s])               # (16, 512) i32
    mask = (sent != PAD_IDX)
    # toks in (w, j, b) order so gathered/psw columns are (j, b)
    toks = sent.reshape(BL, NW, WIN).transpose(1, 2, 0).reshape(T * BL, 1)
    oh = (tags[:, :, None] == np.arange(K)[None, None, :])
    tags1h = (oh & mask[:, :, None]).transpose(2, 1, 0).reshape(K, T * BL)
    tnx = np.zeros((BL, T, K), np.float32)
    tnx[:, :-1, :] = (oh[:, 1:, :] & mask[:, 1:, None]).astype(np.float32)
    m = {
        "toks": toks.astype(np.int32),
        "masku": mask.T.astype(np.uint8).reshape(1, T * BL),
        "tags1h": tags1h.astype(np.uint8),
        "tagsnx": tnx.reshape(T * BL, K).astype(np.float32),
        "tagsfl": tags.reshape(T * BL, 1).astype(np.int32),
        "emb": np.asarray(inputs["embedding"]).astype(dt_np),
        "wih_f": np.ascontiguousarray(_perm_rows(np.asarray(inputs["w_ih_f"])).T).astype(dt_np),
        "wih_b": np.ascontiguousarray(_perm_rows(np.asarray(inputs["w_ih_b"])).T).astype(dt_np),
        "whh_f": np.ascontiguousarray(_perm_rows(np.asarray(inputs["w_hh_f"])).T).astype(dt_np),
        "whh_b": np.ascontiguousarray(_perm_rows(np.asarray(inputs["w_hh_b"])).T).astype(dt_np),
        "bihT6_f": np.ascontiguousarray(_perm_rows(np.asarray(inputs["b_f"])).reshape(8, 128)[0:6]).astype(dt_np),
        "bihT6_b": np.ascontiguousarray(_perm_rows(np.asarray(inputs["b_b"])).reshape(8, 128)[0:6]).astype(dt_np),
        "bihT2_f": np.ascontiguousarray(_perm_rows(np.asarray(inputs["b_f"])).reshape(8, 128)[6:8]).astype(dt_np),
        "bihT2_b": np.ascontiguousarray(_perm_rows(np.asarray(inputs["b_b"])).reshape(8, 128)[6:8]).astype(dt_np),
        "indic6": (np.arange(6)[:, None] == (np.arange(96) // BL)[None, :]).astype(dt_np),
        "indic2": (np.arange(2)[:, None] == (np.arange(32) // BL)[None, :]).astype(dt_np),
        "woutT": np.ascontiguousarray(np.asarray(inputs["w_out"]).T.reshape(4, 128, K)).astype(dt_np),
        "bout": np.asarray(inputs["b_out"]).reshape(K, 1).astype(np.float32),
        "expAT": np.ascontiguousarray(np.exp(np.asarray(inputs["transition"], np.float64)).T).astype(dt_np),
        "expA": np.exp(np.asarray(inputs["transition"], np.float64)).astype(dt_np),
        "trans": np.asarray(inputs["transition"], np.float32),
    }
    return m


def kernel(**inputs):
    import ml_dtypes
    from concourse import mybir
    from concourse.bass_utils import run_bass_kernel_spmd

    use_bf16 = _cache.get("use_bf16", True)
    ml = max(1, int(np.asarray(inputs["sentences_lengths"]).min()))
    key = ("prog", use_bf16, ml)
    if key not in _cache:
        dt_w = mybir.dt.bfloat16 if use_bf16 else mybir.dt.float32
        _cache[key] = _build_program(dt_w, ml)
    nc, names = _cache[key]
    dt_np = ml_dtypes.bfloat16 if use_bf16 else np.float32

    in_maps = []
    for k in range(NCORES):
        m = _prep_core(inputs, k, dt_np)
        in_maps.append({names[kk]: vv for kk, vv in m.items()})

    res = run_bass_kernel_spmd(nc, in_maps, core_ids=list(range(NCORES)),
                               **_cache.get("run_kwargs", {}))
    out = np.concatenate([r[names["out"]].reshape(BL) for r in res.results])
    _cache["last_results"] = res
    return out.astype(np.float32)
